# revision 32
# baseline (speedup 1.0000x reference)
"""Trainium2 Bass kernel for nn_Decoder_20486994002617.  v4.

8-core tensor-parallel 2-layer llama-style decoder with ragged token-merge
(handled on host), returning the masked-mean cross-entropy loss.

v2: fp8e4 DoubleRow for qkv / o / gate / up / down / lm_head, weights
pre-scaled (x64, up-proj x4) into e4m3 range, compensation folded into rope
constants and scaled PSUM->SBUF copies; host pre-chunks weights into SBUF
tile layout so streaming DMAs are contiguous.

v3 (latency restructure): split ARs, bridge xnt preps emitted inside the
previous phase's stream, fixed-max online softmax for the lm head.

v4 (ragged truncation + per-tile collectives):
  - Only the first valid_len tokens of the merged/compacted sequence can
    affect the loss (causal attention + masked CE), so the device kernel
    is compiled for T = ceil(valid_len/128) sequence tiles instead of the
    padded 8 (T=5 for the reference inputs).
  - AllReduces are per-tile (128 rows, ~1MB) and posted as soon as each
    tile's rows are ready, shrinking phase-boundary latency.
  - MLP runs in groups of <=4 tiles (PSUM accumulators); the last group's
    weight streaming hides the next phase's residual+norm+transpose preps.
"""
import numpy as np
import ml_dtypes

from contextlib import ExitStack

import concourse.bass as bass
import concourse.bacc as bacc
import concourse.mybir as mybir
import concourse.tile as tile
from concourse.bass_utils import run_bass_kernel_spmd

F32 = mybir.dt.float32
BF16 = mybir.dt.bfloat16
FP8 = mybir.dt.float8e4
AF = mybir.ActivationFunctionType
ALU = mybir.AluOpType
AX = mybir.AxisListType
DR = mybir.MatmulPerfMode.DoubleRow

H, HD, NH, NKV = 4096, 128, 32, 8
L, V, S, I = 2, 32000, 1024, 11008
EPS, THETA = 1e-6, 10000.0
NC_ = 8          # cores
IPC = I // NC_   # 1376
IP = 1408        # padded intermediate per core = 11 * 128
IP2 = 1536       # fp8-pair-padded contraction for down proj = 12 * 128
VS = V // NC_    # 4000 vocab per core
NEG = -1e9
WS = 64.0        # fp8 weight scale (qkv, o, gate, down, lm_head)
US = 4.0         # fp8 weight scale for y=silu(g)*u (y*US must stay under 240)
LM_MAX = 16.0    # fixed logsumexp shift (|logit| << 16)
SC_MAX = 12.0    # fixed softmax shift for attention scores (|score| << 12)
ALPHA = 16.0     # residual-stream prescale: raw h must stay inside fp8e4
                 # normals (max |h| ~ 11.9 over the run -> 190 < 240)

bf16 = ml_dtypes.bfloat16
f8 = ml_dtypes.float8_e4m3

last_run_info = {}
_cache = {}


# ----------------------------------------------------------------- device --

def _norm_transpose(nc, small, ntmp, psum, h_ap, dst, ident_sb, uid,
                    nt_tag="mix", nt_bufs=2):
    """dst[:, k, :] (32 chunks of [128,128]) = normalized transpose of
    h_ap ([128 seq rows, 4096]). dst free dims must be (32, 128)."""
    ssq = small.tile([128, 1], F32, tag="nt_ssq", bufs=2, name=f"ssq_{uid}")
    # Square scratch output goes into dst (overwritten by the transpose after)
    nc.scalar.activation(dst, h_ap.rearrange("p (k m) -> p k m", k=32),
                         AF.Square, accum_out=ssq[:])
    var = small.tile([128, 1], F32, tag="nt_var", bufs=2, name=f"var_{uid}")
    nc.vector.tensor_scalar(var[:], ssq[:], 1.0 / H, ALPHA * ALPHA * EPS,
                            op0=ALU.mult, op1=ALU.add)
    std = small.tile([128, 1], F32, tag="nt_std", bufs=2, name=f"std_{uid}")
    nc.scalar.sqrt(std[:], var[:])
    fac = small.tile([128, 1], F32, tag="nt_fac", bufs=2, name=f"fac_{uid}")
    nc.vector.reciprocal(fac[:], std[:])
    diag = ntmp.tile([128, 128], BF16, tag="nt_diag", bufs=2, name=f"diag_{uid}")
    nc.vector.tensor_scalar_mul(diag[:], ident_sb[:], fac[:])
    for kk in range(8):
        pnt = psum.tile([128, 512], F32, tag=nt_tag, bufs=nt_bufs,
                        name=f"pnt_{uid}_{kk}")
        for j in range(4):
            k = kk * 4 + j
            nc.tensor.matmul(pnt[:, j * 128:(j + 1) * 128],
                             h_ap[:, k * 128:(k + 1) * 128], diag[:],
                             start=True, stop=True)
        nc.any.tensor_copy(dst[:, kk * 4:(kk + 1) * 4, :],
                           pnt[:].rearrange("p (j m) -> p j m", j=4))


def _transpose_raw(nc, psum, h_ap, dst, ident_sb, uid, nt_tag="mix", nt_bufs=2):
    """dst[:, k, :] (32 chunks of [128,128]) = raw transpose of h_ap
    ([128 seq rows, 4096]); per-row norm factors are folded into the
    consumers' outputs instead (no vector dependency here)."""
    for kk in range(8):
        pnt = psum.tile([128, 512], F32, tag=nt_tag, bufs=nt_bufs,
                        name=f"pnt_{uid}_{kk}")
        for j in range(4):
            k = kk * 4 + j
            nc.tensor.matmul(pnt[:, j * 128:(j + 1) * 128],
                             h_ap[:, k * 128:(k + 1) * 128], ident_sb[:],
                             start=True, stop=True)
        nc.any.tensor_copy(dst[:, kk * 4:(kk + 1) * 4, :],
                           pnt[:].rearrange("p (j m) -> p j m", j=4))


def _norm_stats(nc, small, scrpool, h_ap, fac_ap, facws_ap, uid):
    """fac = 1/sqrt(mean(h^2)/1 + a^2 eps) per row; facws = fac/WS."""
    scr = scrpool.tile([128, 32, 128], BF16, tag="nt_scr", bufs=2,
                       name=f"scr_{uid}")
    ssq = small.tile([128, 1], F32, tag="nt_ssq", bufs=2, name=f"ssq_{uid}")
    nc.scalar.activation(scr[:], h_ap.rearrange("p (k m) -> p k m", k=32),
                         AF.Square, accum_out=ssq[:])
    var = small.tile([128, 1], F32, tag="nt_var", bufs=2, name=f"var_{uid}")
    nc.gpsimd.tensor_scalar(var[:], ssq[:], 1.0 / H, ALPHA * ALPHA * EPS,
                            op0=ALU.mult, op1=ALU.add)
    std = small.tile([128, 1], F32, tag="nt_std", bufs=2, name=f"std_{uid}")
    nc.scalar.sqrt(std[:], var[:])
    nc.vector.reciprocal(fac_ap, std[:])
    nc.gpsimd.tensor_scalar_mul(facws_ap, fac_ap, 1.0 / WS)


def _rope(nc, ntmp, ps, cos_ap, sf_ap, out, nheads, i):
    """out (bf16 [128, nheads*128]) = rope(ps); cos_ap/sf_ap are [128,128]."""
    n = nheads * 128
    t1 = ntmp.tile([128, 512], F32, tag="rope_t1", bufs=1, name=f"t1_{i}_{nheads}")
    t2 = ntmp.tile([128, 512], F32, tag="rope_t2", bufs=1, name=f"t2_{i}_{nheads}")
    for hh in range(nheads):
        b = hh * 128
        nc.vector.tensor_mul(t1[:, b:b + 128], ps[:, b:b + 128], cos_ap)
        nc.vector.tensor_mul(t2[:, b:b + 64], ps[:, b + 64:b + 128],
                             sf_ap[:, 0:64])
        nc.vector.tensor_mul(t2[:, b + 64:b + 128], ps[:, b:b + 64],
                             sf_ap[:, 64:128])
    nc.vector.tensor_add(out[:], t1[:, :n], t2[:, :n])


def build_nc(T):
    VLP = T * 128
    # MLP tile groups: keep the last group >=2 tiles so its re-streamed
    # weights amortize over enough GEMM work (compute-bound, not DMA-bound)
    split = min(4, max(1, T - 2))
    g0 = list(range(split))              # first MLP group (<=4 tiles)
    glast = list(range(split, T))        # last MLP group ([] if T <= 1)
    groups = [g0] + ([glast] if glast else [])

    nc = bacc.Bacc("TRN2", target_bir_lowering=False, debug=False,
                   num_devices=NC_)

    din = {}
    def dram_in(name, shape, dtype=BF16):
        din[name] = nc.dram_tensor(name, shape, dtype, kind="ExternalInput")
        return din[name]

    h0_d = dram_in("h0", [VLP, H])
    cos1_d = dram_in("cos1", [VLP, 128])
    sf1_d = dram_in("sf1", [VLP, 128])
    ident_d = dram_in("ident", [128, 128])
    cmask_d = dram_in("cmask", [128, 128])
    ones_d = dram_in("ones", [128, 1])
    for l in range(L):
        dram_in(f"qkvw{l}", [128, 32, 768], FP8)
        dram_in(f"ow{l}", [128, 4, H], FP8)
        dram_in(f"gw{l}", [3, 8, 128, 4, 512], FP8)   # [nb, kp, p, j, n]
        dram_in(f"uw{l}", [3, 8, 128, 4, 512], FP8)
        dram_in(f"dw{l}", [8, 3, 128, 4, 512], FP8)   # [n, tp, p, j, n]
    lmw_d = dram_in("lmw", [8, 8, 128, 4, 500], FP8)  # [vb, kp, p, j, n]
    wsel_d = dram_in("wsel", [H, VLP])

    gsum_o = nc.dram_tensor("gsum_o", [128, 2 * T], F32, kind="ExternalOutput")
    tlog_o = nc.dram_tensor("tlog_o", [1, VLP], F32, kind="ExternalOutput")

    rg = [list(range(NC_))]

    with tile.TileContext(nc) as tc:
        with (
            tc.tile_pool(name="pconst", bufs=1) as pconst,
            tc.tile_pool(name="psmall", bufs=1) as psmall,
            tc.tile_pool(name="pbridge", bufs=1) as pbridge,
            tc.tile_pool(name="pdram", bufs=1, space="DRAM") as pdram,
        ):
            ident_sb = pconst.tile([128, 128], BF16)
            cmask_sb = pconst.tile([128, 128], BF16)
            ones_sb = pconst.tile([128, 1], BF16)
            cos_sb = pconst.tile([128, T, 128], BF16)
            sf_sb = pconst.tile([128, T, 128], BF16)
            negSM = pconst.tile([128, 1], F32)
            nc.any.memset(negSM[:], -SC_MAX)
            nc.sync.dma_start(ident_sb[:], ident_d.ap())
            nc.sync.dma_start(cmask_sb[:], cmask_d.ap())
            nc.sync.dma_start(ones_sb[:], ones_d.ap())
            for i in range(T):
                nc.sync.dma_start(cos_sb[:, i, :], cos1_d.ap()[i * 128:(i + 1) * 128, :])
                nc.sync.dma_start(sf_sb[:, i, :], sf1_d.ap()[i * 128:(i + 1) * 128, :])

            xnt = [pbridge.tile([128, 32, 128], FP8, name=f"xnt_{j}")
                   for j in range(T)]
            fac_sbs = [pbridge.tile([128, 1], F32, name=f"fac_{j}")
                       for j in range(T)]
            facws_sbs = [pbridge.tile([128, 1], F32, name=f"facws_{j}")
                         for j in range(T)]

            hstack = ExitStack()
            phh = hstack.enter_context(tc.tile_pool(name="phh", bufs=1))
            h_sb = phh.tile([128, T, H], BF16)
            for i in range(T):
                nc.sync.dma_start(h_sb[:, i, :], h0_d.ap()[i * 128:(i + 1) * 128, :])

            # per-tile AR buffers: [T][128, H]
            ar_ins, ar_outss, ar2_ins, ar2_outss = [], [], [], []
            for l in range(L):
                ar_ins.append(pdram.tile([VLP, H], BF16, name=f"ar_in_{l}"))
                ar_outss.append([pdram.tile([128, H], BF16, addr_space="Shared",
                                            name=f"ar_out_{l}_{j}")
                                 for j in range(T)])
                ar2_ins.append(pdram.tile([VLP, H], BF16, name=f"ar2_in_{l}"))
                ar2_outss.append([pdram.tile([128, H], BF16, addr_space="Shared",
                                             name=f"ar2_out_{l}_{j}")
                                  for j in range(T)])

            def prep(pool, psum, j, res_t, dst, uid, nt_tag="mix", nt_bufs=2,
                     norm=False):
                """h_sb[:,j] += AR per-tile residual; dst = transpose.
                norm=True bakes the rms factor into dst (final xf path);
                otherwise dst is raw and fac/facws are computed on the side."""
                if res_t is not None:
                    rt = pool.tile([128, H], BF16, tag="prep_rt", bufs=2,
                                   name=f"rt_{uid}")
                    nc.sync.dma_start(rt[:], res_t[:])
                    nc.vector.tensor_add(h_sb[:, j, :], h_sb[:, j, :], rt[:])
                if norm:
                    _norm_transpose(nc, psmall, pool, psum, h_sb[:, j, :], dst,
                                    ident_sb, uid, nt_tag=nt_tag, nt_bufs=nt_bufs)
                else:
                    _transpose_raw(nc, psum, h_sb[:, j, :], dst, ident_sb,
                                   uid, nt_tag=nt_tag, nt_bufs=nt_bufs)
                    _norm_stats(nc, psmall, pool, h_sb[:, j, :],
                                fac_sbs[j][:], facws_sbs[j][:], uid)

            xfstack = ExitStack()
            xf_sb = None

            for l in range(L):
                # ======== attention: per-tile qkv -> heads -> o-proj ========
                with (
                    tc.tile_pool(name="pal", bufs=1) as pal,
                    tc.tile_pool(name="paps", bufs=1, space="PSUM") as paps,
                ):
                    kT_sb = pal.tile([128, VLP], BF16)
                    v_sb = pal.tile([128, T, 128], BF16)
                    ar_in = ar_ins[l]
                    ar_outs = ar_outss[l]
                    wqkv_sb = pal.tile([128, 32, 768], FP8)
                    ow_sb = pal.tile([128, 4, H], FP8)
                    nc.sync.dma_start(wqkv_sb[:], din[f"qkvw{l}"].ap())
                    nc.sync.dma_start(ow_sb[:], din[f"ow{l}"].ap())
                    if l == 0:
                        for j in range(T):
                            prep(pal, paps, j, None, xnt[j], f"i{j}")
                        late = []
                    else:
                        # tiles of the previous MLP's last group could not be
                        # prepped there (their ar2 posts at its very end)
                        late = list(glast)
                    qTs = {}

                    def emit_A(i):
                        """qkv GEMM + rope + transposes for tile i (no
                        dependence on any other tile's attention)."""
                        psq_t = paps.tile([128, 512], F32, tag="psq", bufs=2,
                                          name=f"psq_{l}_{i}")
                        pskv_t = paps.tile([128, 256], F32, tag="pskv", bufs=1,
                                           name=f"pskv_{l}_{i}")
                        psq = psq_t[:]
                        pskv = pskv_t[:]
                        for k in range(16):
                            nc.tensor.matmul(psq, xnt[i][:, 2 * k:2 * k + 2, :],
                                             wqkv_sb[:, 2 * k:2 * k + 2, 0:512],
                                             start=(k == 0), stop=(k == 15),
                                             perf_mode=DR)
                            nc.tensor.matmul(pskv, xnt[i][:, 2 * k:2 * k + 2, :],
                                             wqkv_sb[:, 2 * k:2 * k + 2, 512:768],
                                             start=(k == 0), stop=(k == 15),
                                             perf_mode=DR)
                        qT_sb = pal.tile([128, 4, 128], BF16, tag="qT",
                                         bufs=2, name=f"qT_{l}_{i}")
                        qTs[i] = qT_sb
                        q_rot = pal.tile([128, 512], BF16, tag="q_rot", bufs=2,
                                         name=f"qr_{l}_{i}")
                        k_rot = pal.tile([128, 128], BF16, tag="k_rot", bufs=2,
                                         name=f"kr_{l}_{i}")
                        _rope(nc, pal, psq, cos_sb[:, i, :], sf_sb[:, i, :],
                              q_rot, 4, f"{l}_{i}")
                        nc.vector.tensor_scalar_mul(q_rot[:], q_rot[:],
                                                    fac_sbs[i][:])
                        _rope(nc, pal, pskv_t[:, 0:128], cos_sb[:, i, :],
                              sf_sb[:, i, :], k_rot, 1, f"{l}_{i}")
                        nc.vector.tensor_scalar_mul(k_rot[:], k_rot[:],
                                                    fac_sbs[i][:])
                        nc.vector.tensor_scalar_mul(v_sb[:, i, :],
                                                    pskv_t[:, 128:256],
                                                    facws_sbs[i][:])
                        ptr = paps.tile([128, 512], F32, tag="mix", bufs=2,
                                        name=f"ptrq_{l}_{i}")
                        for hh in range(4):
                            nc.tensor.matmul(ptr[:, hh * 128:(hh + 1) * 128],
                                             q_rot[:, hh * 128:(hh + 1) * 128],
                                             ident_sb[:], start=True, stop=True)
                        nc.any.tensor_copy(qT_sb[:],
                                           ptr[:].rearrange("p (j m) -> p j m", j=4))
                        ptrk = paps.tile([128, 512], F32, tag="mix", bufs=2,
                                         name=f"ptrk_{l}_{i}")
                        nc.tensor.matmul(ptrk[:, :128], k_rot[:], ident_sb[:],
                                         start=True, stop=True)
                        nc.any.tensor_copy(kT_sb[:, i * 128:(i + 1) * 128], ptrk[:, :128])

                    emit_A(0)
                    for i in range(T):
                        if i + 1 < T:
                            emit_A(i + 1)
                        qT_sb = qTs.pop(i)
                        oT_sb = pal.tile([128, 4, 128], FP8, tag="oT",
                                         bufs=2, name=f"oT_{l}_{i}")
                        n2 = 128 * (i + 1)
                        for hh in range(4):
                            pss = paps.tile([128, VLP], F32, tag="pss", bufs=1,
                                            name=f"pss_{l}_{hh}_{i}")
                            lhs_q = qT_sb[:, hh, :]
                            c0 = 0
                            while c0 < n2 - 128:
                                N = min(512, n2 - 128 - c0)
                                nc.tensor.matmul(pss[:, c0:c0 + N], lhs_q,
                                                 kT_sb[:, c0:c0 + N],
                                                 start=True, stop=True)
                                c0 += N
                            nc.tensor.matmul(pss[:, n2 - 128:n2], lhs_q,
                                             kT_sb[:, n2 - 128:n2],
                                             start=True, stop=False)
                            nc.tensor.matmul(pss[:, n2 - 128:n2], ident_sb[:],
                                             cmask_sb[:], start=False, stop=True)
                            sume = psmall.tile([128, 1], F32, tag="sume", bufs=2,
                                               name=f"sume_{l}_{hh}_{i}")
                            exp_sb = pal.tile([128, VLP], BF16, tag="exp", bufs=2,
                                              name=f"exp_{l}_{hh}_{i}")
                            nc.scalar.activation(exp_sb[:, :n2], pss[:, :n2], AF.Exp,
                                                 bias=negSM[:], accum_out=sume[:])
                            rec = psmall.tile([128, 1], F32, tag="rec", bufs=2,
                                              name=f"rec_{l}_{hh}_{i}")
                            nc.vector.reciprocal(rec[:], sume[:])
                            diag_r = pal.tile([128, 128], BF16, tag="diag_r", bufs=2,
                                              name=f"diagr_{l}_{hh}_{i}")
                            nc.vector.tensor_scalar_mul(diag_r[:], ident_sb[:], rec[:])
                            atcol = pal.tile([128, T, 128], BF16, tag="atcol", bufs=2,
                                             name=f"atcol_{l}_{hh}_{i}")
                            for jg in range(0, i + 1, 4):
                                ng = min(4, i + 1 - jg)
                                pat = paps.tile([128, 512], F32, tag="mix", bufs=2,
                                                name=f"pat_{l}_{hh}_{i}_{jg}")
                                for jj in range(ng):
                                    j = jg + jj
                                    nc.tensor.matmul(pat[:, jj * 128:(jj + 1) * 128],
                                                     exp_sb[:, j * 128:(j + 1) * 128],
                                                     diag_r[:], start=True, stop=True)
                                nc.any.tensor_copy(
                                    atcol[:, jg:jg + ng, :],
                                    pat[:, :ng * 128].rearrange("p (j m) -> p j m", j=ng))
                            pso = paps.tile([128, 128], F32, tag="pso", bufs=1,
                                            name=f"pso_{l}_{hh}_{i}")
                            for j in range(i + 1):
                                nc.tensor.matmul(pso[:], v_sb[:, j, :], atcol[:, j, :],
                                                 start=(j == 0), stop=(j == i))
                            nc.any.tensor_copy(oT_sb[:, hh, :], pso[:])
                        ob = pal.tile([128, H], BF16, tag="ob", bufs=1,
                                      name=f"ob_{l}_{i}")
                        for n in range(8):
                            pps = paps.tile([128, 512], F32, tag="mix", bufs=2,
                                            name=f"pop_{l}_{i}_{n}")
                            for t in range(2):
                                nc.tensor.matmul(pps[:], oT_sb[:, 2 * t:2 * t + 2, :],
                                                 ow_sb[:, 2 * t:2 * t + 2,
                                                       n * 512:(n + 1) * 512],
                                                 start=(t == 0), stop=(t == 1),
                                                 perf_mode=DR)
                            nc.vector.tensor_scalar_mul(ob[:, n * 512:(n + 1) * 512],
                                                        pps[:], 1.0 / WS)
                        nc.sync.dma_start(ar_in[i * 128:(i + 1) * 128, :], ob[:])
                        nc.gpsimd.collective_compute(
                            "AllReduce", ALU.add, replica_groups=rg,
                            ins=[ar_in[i * 128:(i + 1) * 128, :].opt()],
                            outs=[ar_outs[i].opt()])
                        # prep this phase's late tiles (prev MLP tail ARs)
                        if l > 0 and 1 <= i <= len(late):
                            j = late[i - 1]
                            prep(pal, paps, j, ar2_outss[l - 1][j], xnt[j],
                                 f"a{l}_{j}")
                        # prep the first MLP tiles as their ARs land
                        if i == T - 2 and len(g0) > 0:
                            prep(pal, paps, 0, ar_outs[0], xnt[0], f"m{l}_0")
                        if i == T - 1 and len(g0) > 1:
                            prep(pal, paps, 1, ar_outs[1], xnt[1], f"m{l}_1")
                    for j in g0[2:]:
                        prep(pal, paps, j, ar_outs[j], xnt[j], f"m{l}_{j}")

                # ===== MLP: gate/up -> down in tile groups -> AR2 ===========
                if l == L - 1:
                    pxf = xfstack.enter_context(tc.tile_pool(name="pxf", bufs=1))
                    xf_sb = pxf.tile([128, 32, VLP], FP8)
                with (
                    tc.tile_pool(name="pml", bufs=1) as pml,
                    tc.tile_pool(name="pmps", bufs=1, space="PSUM") as pmps,
                ):
                    ar2_in = ar2_ins[l]
                    ar2_outs = ar2_outss[l]

                    def prep_next(j):
                        """prep for the next phase: attention l+1 (into xnt)
                        or the final norm (into xf)."""
                        if l < L - 1:
                            prep(pml, pmps, j, ar2_outs[j], xnt[j],
                                 f"a{l + 1}_{j}", nt_tag="mlpps", nt_bufs=4)
                        else:
                            prep(pml, pmps, j, ar2_outs[j],
                                 xf_sb[:, :, j * 128:(j + 1) * 128],
                                 f"f{j}", nt_tag="mlpps", nt_bufs=4, norm=True)

                    for gi, tiles_g in enumerate(groups):
                        glen = len(tiles_g)
                        # slots: work to interleave into this group's weight
                        # streaming (one item consumed per g/u nb-block)
                        if gi == 0 and glast:
                            # this phase's last-group tiles (attn ARs ready)
                            slots = [(j, ar_outss[l][j], xnt[j], f"m{l}_{j}")
                                     for j in glast]
                        elif gi == len(groups) - 1 and glast:
                            # next phase's first-group tiles (ar2 of group 0)
                            slots = [("next", j) for j in g0]
                        else:
                            slots = []
                        slots = list(slots)

                        def pop_slot():
                            if slots:
                                s = slots.pop(0)
                                if s[0] == "next":
                                    prep_next(s[1])
                                else:
                                    j, res_t, dst, uid = s
                                    prep(pml, pmps, j, res_t, dst, uid,
                                         nt_tag="mlpps", nt_bufs=4)

                        with tc.tile_pool(name=f"pgu{gi}", bufs=1) as pgu:
                            yt_sb = pml.tile([128, 12, glen * 128], FP8, tag="yt",
                                             bufs=2, name=f"yt_{l}_{gi}")
                            nc.gpsimd.memset(yt_sb[:, 11, :], 0.0)
                            gu = {}
                            for wi, (wname, tag) in enumerate(
                                    ((f"gw{l}", "g"), (f"uw{l}", "u"))):
                                outs = [pgu.tile([128, IP], BF16, tag=tag, bufs=4,
                                                 name=f"{tag}_{l}_{gi}_{ii}")
                                        for ii in range(glen)]
                                gu[tag] = outs
                                for nb in range(3):
                                    NB = 512 if nb < 2 else IP - 1024
                                    pg = [pmps.tile([128, 512], F32, tag="mlpps", bufs=4,
                                                    name=f"pg_{l}_{gi}_{tag}_{nb}_{ii}")
                                          for ii in range(glen)]
                                    for kp in range(8):
                                        wt = pgu.tile([128, 4, 512], FP8, tag="wstream",
                                                      bufs=6,
                                                      name=f"wt_{l}_{gi}_{tag}_{nb}_{kp}")
                                        nc.sync.dma_start(wt[:], din[wname].ap()[nb, kp])
                                        for jp in range(2):
                                            k = kp * 4 + 2 * jp
                                            for ii in range(glen):
                                                nc.tensor.matmul(
                                                    pg[ii][:, :NB],
                                                    xnt[tiles_g[ii]][:, k:k + 2, :],
                                                    wt[:, 2 * jp:2 * jp + 2, :NB],
                                                    start=(k == 0), stop=(k == 30),
                                                    perf_mode=DR)
                                    for ii in range(glen):
                                        nc.vector.tensor_scalar_mul(
                                            outs[ii][:, nb * 512:nb * 512 + NB],
                                            pg[ii][:, :NB],
                                            fac_sbs[tiles_g[ii]][:])
                                    if not (wi == 0 and nb == 0):
                                        pop_slot()
                            pairs = [tiles_g[k:k + 2]
                                     for k in range(0, glen, 2)]
                            for iq, pair in enumerate(pairs):
                                for ii in pair:
                                    loc = tiles_g.index(ii)
                                    ysil = pgu.tile([128, IP], BF16, tag="ysil", bufs=2,
                                                    name=f"ysil_{l}_{ii}")
                                    nc.scalar.activation(ysil[:], gu["g"][loc][:], AF.Silu,
                                                         scale=1.0 / WS)
                                    y = gu["u"][loc]
                                    nc.vector.tensor_mul(y[:], ysil[:], y[:])
                                    for tq in range(3):
                                        ts = [tq * 4 + j for j in range(4) if tq * 4 + j < 11]
                                        ptr = pmps.tile([128, 512], F32, tag="mlpps", bufs=4,
                                                        name=f"ytr_{l}_{ii}_{tq}")
                                        for jj, t in enumerate(ts):
                                            nc.tensor.matmul(ptr[:, jj * 128:(jj + 1) * 128],
                                                             y[:, t * 128:(t + 1) * 128],
                                                             ident_sb[:], start=True, stop=True)
                                        nc.any.tensor_copy(
                                            yt_sb[:, ts[0]:ts[0] + len(ts),
                                                  loc * 128:(loc + 1) * 128],
                                            ptr[:, :len(ts) * 128].rearrange(
                                                "p (j m) -> p j m", j=len(ts)))
                                for n in range(8):
                                    pd = [pmps.tile([128, 512], F32, tag=f"pd{i2}", bufs=1,
                                                    name=f"pd_{l}_{gi}_{iq}_{n}_{i2}")
                                          for i2 in range(len(pair))]
                                    for tp in range(3):
                                        dwt = pgu.tile([128, 4, 512], FP8, tag="dwstream",
                                                       bufs=6,
                                                       name=f"dwt_{l}_{gi}_{iq}_{n}_{tp}")
                                        nc.sync.dma_start(dwt[:], din[f"dw{l}"].ap()[n, tp])
                                        for jp in range(2):
                                            c = tp * 4 + 2 * jp
                                            for i2, ii in enumerate(pair):
                                                loc = tiles_g.index(ii)
                                                nc.tensor.matmul(
                                                    pd[i2][:],
                                                    yt_sb[:, c:c + 2,
                                                          loc * 128:(loc + 1) * 128],
                                                    dwt[:, 2 * jp:2 * jp + 2, :],
                                                    start=(c == 0), stop=(c == 10),
                                                    perf_mode=DR)
                                    for i2, ii in enumerate(pair):
                                        db = pgu.tile([128, 512], BF16, tag="db", bufs=2,
                                                      name=f"db_{l}_{gi}_{iq}_{n}_{i2}")
                                        nc.vector.tensor_scalar_mul(db[:], pd[i2][:],
                                                                    1.0 / (WS * US))
                                        nc.sync.dma_start(
                                            ar2_in[ii * 128:(ii + 1) * 128,
                                                   n * 512:(n + 1) * 512], db[:])
                                for ii in pair:
                                    nc.gpsimd.collective_compute(
                                        "AllReduce", ALU.add, replica_groups=rg,
                                        ins=[ar2_in[ii * 128:(ii + 1) * 128, :].opt()],
                                        outs=[ar2_outs[ii].opt()])
                        if not glast and gi == len(groups) - 1:
                            # single-group fallback: emit all next-phase preps
                            for j in range(T):
                                prep_next(j)

                    if l == L - 1:
                        # final xf tiles of the last group (their ar2 lands
                        # late) with the tlog groups interleaved to hide the
                        # tail AR latency
                        with (
                            tc.tile_pool(name="ptl", bufs=1) as ptl,
                            tc.tile_pool(name="ptps", bufs=1, space="PSUM") as ptps,
                        ):
                            WA = len(g0) * 128
                            widths = [WA] + ([VLP - WA] if VLP > WA else [])
                            pts = [ptps.tile([1, w], F32, name=f"pt{h_}")
                                   for h_, w in enumerate(widths)]

                            def tlog_part(part, c0, W):
                                for kp in range(8):
                                    ws = ptl.tile([128, 4, W], BF16, tag="wsel",
                                                  bufs=2, name=f"ws_{part}_{kp}")
                                    nc.sync.dma_start(
                                        ws[:], wsel_d.ap()[kp * 512:(kp + 1) * 512,
                                                           c0:c0 + W]
                                        .rearrange("(j p) n -> p j n", p=128))
                                    for jk in range(4):
                                        k = kp * 4 + jk
                                        tm = ptl.tile([128, W], BF16, tag="tm",
                                                      bufs=2, name=f"tm_{part}_{k}")
                                        nc.vector.tensor_mul(
                                            tm[:], xf_sb[:, k, c0:c0 + W],
                                            ws[:, jk, :])
                                        nc.tensor.matmul(pts[part][:], ones_sb[:],
                                                         tm[:], start=(k == 0),
                                                         stop=(k == 31))

                            tlog_part(0, 0, WA)
                            for j in glast:
                                prep_next(j)
                            if len(widths) > 1:
                                tlog_part(1, WA, VLP - WA)
                            tl_sb = ptl.tile([1, VLP], F32)
                            nc.any.tensor_copy(tl_sb[:, :WA], pts[0][:])
                            if len(widths) > 1:
                                nc.any.tensor_copy(tl_sb[:, WA:], pts[1][:])
                            nc.sync.dma_start(tlog_o.ap(), tl_sb[:])

            # ==================== lm (online fixed-max softmax) =============

            with (
                tc.tile_pool(name="plm", bufs=1) as plm,
                tc.tile_pool(name="plps", bufs=1, space="PSUM") as plps,
                tc.tile_pool(name="pld", bufs=1, space="DRAM") as pld,
            ):
                # two partial-sum accumulators so the first AR (vocab blocks
                # 0..5) overlaps the last two blocks' compute
                s_sbs = [plm.tile([128, T], F32, name=f"s_sb{p}")
                         for p in range(2)]
                for p in range(2):
                    nc.any.memset(s_sbs[p][:], 0.0)
                negM = plm.tile([128, 1], F32)
                nc.any.memset(negM[:], -LM_MAX)
                gs_ins = [pld.tile([128, T], F32, name=f"gs_in{p}")
                          for p in range(2)]
                gs_outs = [pld.tile([128, T], F32, addr_space="Shared",
                                    name=f"gs_out{p}") for p in range(2)]
                for vb in range(8):
                    part = 0 if vb < 6 else 1
                    pl = [plps.tile([128, 500], F32, tag=f"pl{i}", bufs=1,
                                    name=f"pl_{vb}_{i}") for i in range(T)]
                    for kp in range(8):
                        lt = plm.tile([128, 4, 500], FP8, tag="lmw", bufs=6,
                                      name=f"lt_{vb}_{kp}")
                        nc.sync.dma_start(lt[:], lmw_d.ap()[vb, kp])
                        for jp in range(2):
                            k = kp * 4 + 2 * jp
                            for i in range(T):
                                nc.tensor.matmul(pl[i][:],
                                                 xf_sb[:, k:k + 2, i * 128:(i + 1) * 128],
                                                 lt[:, 2 * jp:2 * jp + 2, :],
                                                 start=(k == 0), stop=(k == 30),
                                                 perf_mode=DR)
                    for i in range(T):
                        se = psmall.tile([128, 1], F32, tag="se", bufs=2,
                                         name=f"se_{vb}_{i}")
                        scr = plm.tile([128, 500], BF16, tag="scr", bufs=2,
                                       name=f"scr_{vb}_{i}")
                        nc.scalar.activation(scr[:], pl[i][:], AF.Exp,
                                             bias=negM[:], scale=1.0 / WS,
                                             accum_out=se[:])
                        nc.gpsimd.tensor_add(s_sbs[part][:, i:i + 1],
                                             s_sbs[part][:, i:i + 1], se[:])
                    if vb == 5 or vb == 7:
                        p = 0 if vb == 5 else 1
                        nc.sync.dma_start(gs_ins[p][:], s_sbs[p][:])
                        nc.gpsimd.collective_compute(
                            "AllReduce", ALU.add, replica_groups=rg,
                            ins=[gs_ins[p].opt()], outs=[gs_outs[p].opt()])
                gsf_sb = plm.tile([128, 2, T], F32)
                for p in range(2):
                    nc.sync.dma_start(gsf_sb[:, p, :], gs_outs[p][:])
                nc.sync.dma_start(gsum_o.ap(), gsf_sb[:])
            xfstack.close()
            hstack.close()

    nc.compile()
    return nc


# ------------------------------------------------------------------- host --

def _to_f8(x):
    return np.clip(x, -240.0, 240.0).astype(f8)


def host_prep(inputs):
    inp = {k: np.asarray(v) for k, v in inputs.items()}
    embed = inp["embed"].astype(np.float32)
    ids = inp["input_ids"].reshape(-1).astype(np.int64)
    labels = inp["labels"].reshape(-1).astype(np.int64)

    h = embed[ids]
    cw = inp["conv_w"].astype(np.float32)
    logit = h[:-1] @ cw[0, :H] + h[1:] @ cw[0, H:] + np.float32(inp["conv_b"][0])
    mask = logit > 0
    m = np.concatenate([mask, [False]])
    hn = np.where(m[:, None], 0.5 * (h + np.roll(h, -1, axis=0)), h)
    keep = np.concatenate([[True], ~mask])
    order = np.argsort(~keep, kind="stable")
    h0 = hn[order]
    lab = labels[order]
    valid_len = int(keep.sum())

    # ragged truncation: tokens >= valid_len cannot affect the masked loss
    T = (valid_len + 127) // 128
    VLP = T * 128
    h0 = h0[:VLP]

    inv = 1.0 / (THETA ** (np.arange(0, HD, 2, dtype=np.float32) / HD))
    t = np.arange(VLP, dtype=np.float32)
    freqs = np.outer(t, inv)
    emb = np.concatenate([freqs, freqs], -1)
    cos, sin = np.cos(emb), np.sin(emb)
    sinflip = np.concatenate([-sin[:, :HD // 2], sin[:, HD // 2:]], -1)
    # rope constants absorb the 1/WS compensation for the fp8 q/k weights
    cos1 = (cos / WS).astype(bf16)
    sf1 = (sinflip / WS).astype(bf16)

    ident = np.eye(128, dtype=bf16)
    cmask = np.where(np.arange(128)[None, :] > np.arange(128)[:, None],
                     np.float32(NEG), np.float32(0)).astype(bf16)
    ones = np.ones((128, 1), dtype=bf16)

    ln1 = inp["ln1_w"].astype(np.float32)
    ln2 = inp["ln2_w"].astype(np.float32)
    normw = inp["norm_w"].astype(np.float32)
    qsc = np.float32(1.0 / np.sqrt(HD))
    lm_folded = normw[:, None] * inp["lm_head_w"].astype(np.float32)
    lm_q = _to_f8(lm_folded * WS)          # quantized once, reused for wsel
    tgt = np.concatenate([lab[1:], [0]]).astype(np.int64)[:VLP]
    wsel = np.ascontiguousarray(lm_q.astype(np.float32)[:, tgt] / WS).astype(bf16)

    common = dict(h0=(h0 * ALPHA).astype(bf16), cos1=cos1, sf1=sf1,
                  ident=ident, cmask=cmask, ones=ones, wsel=wsel)
    in_maps = []
    for c in range(NC_):
        mcore = dict(common)
        for l in range(L):
            qw = ln1[l][:, None] * inp["q_w"][l].astype(np.float32) * qsc * WS
            kw = ln1[l][:, None] * inp["k_w"][l].astype(np.float32) * WS
            vw = ln1[l][:, None] * inp["v_w"][l].astype(np.float32) * WS
            gw = ln2[l][:, None] * inp["gate_w"][l].astype(np.float32) * WS
            uw = ln2[l][:, None] * inp["up_w"][l].astype(np.float32) * US
            dw = inp["down_w"][l].astype(np.float32) * (WS * ALPHA)
            qkv = np.concatenate(
                [qw[:, c * 512:(c + 1) * 512],
                 kw[:, c * 128:(c + 1) * 128],
                 vw[:, c * 128:(c + 1) * 128]], 1)          # [H, 768]
            mcore[f"qkvw{l}"] = np.ascontiguousarray(
                _to_f8(qkv).reshape(32, 128, 768).transpose(1, 0, 2))
            ow = inp["o_w"][l][c * 512:(c + 1) * 512].astype(np.float32) * (WS * ALPHA)
            mcore[f"ow{l}"] = np.ascontiguousarray(
                _to_f8(ow).reshape(4, 128, H).transpose(1, 0, 2))
            gws = np.zeros((H, IP), np.float32)
            uws = np.zeros((H, IP), np.float32)
            dws = np.zeros((IP2, H), np.float32)
            gws[:, :IPC] = gw[:, c * IPC:(c + 1) * IPC]
            uws[:, :IPC] = uw[:, c * IPC:(c + 1) * IPC]
            dws[:IPC] = dw[c * IPC:(c + 1) * IPC]
            for wname, warr in ((f"gw{l}", gws), (f"uw{l}", uws)):
                out = np.zeros((3, 8, 128, 4, 512), np.float32)
                for nb in range(3):
                    NBc = 512 if nb < 2 else IP - 1024
                    blk = warr[:, nb * 512:nb * 512 + NBc]       # [H, NBc]
                    out[nb, :, :, :, :NBc] = blk.reshape(
                        8, 4, 128, NBc).transpose(0, 2, 1, 3)
                mcore[wname] = _to_f8(out)
            dout = np.zeros((8, 3, 128, 4, 512), np.float32)
            for n in range(8):
                blk = dws[:, n * 512:(n + 1) * 512]              # [IP2, 512]
                dout[n] = blk.reshape(3, 4, 128, 512).transpose(0, 2, 1, 3)
            mcore[f"dw{l}"] = _to_f8(dout)
        lmc = lm_q[:, c * VS:(c + 1) * VS].astype(np.float32)    # [H, 4000]
        lout = np.zeros((8, 8, 128, 4, 500), np.float32)
        for vb in range(8):
            blk = lmc[:, vb * 500:(vb + 1) * 500]                # [H, 500]
            lout[vb] = blk.reshape(8, 4, 128, 500).transpose(0, 2, 1, 3)
        mcore["lmw"] = _to_f8(lout)
        in_maps.append(mcore)

    return in_maps, valid_len, T


def kernel(**inputs) -> np.ndarray:
    in_maps, valid_len, T = host_prep(inputs)
    VLP = T * 128
    if T not in _cache:
        _cache[T] = build_nc(T)
    nc = _cache[T]
    res = run_bass_kernel_spmd(nc, in_maps, list(range(NC_)),
                               **last_run_info.get("run_kwargs", {}))
    last_run_info["res"] = res
    out = res.results[0]
    gs = out["gsum_o"].reshape(128, 2, T).astype(np.float64)
    gsum = (gs[:, 0, :] + gs[:, 1, :]).transpose(1, 0).reshape(VLP)
    tlog = out["tlog_o"].reshape(VLP).astype(np.float64)
    ce = LM_MAX + np.log(gsum) - tlog
    loss = ce[:valid_len - 1].sum() / (valid_len - 1)
    return np.float32(loss)


# revision 33
# speedup vs baseline: 1.0215x; 1.0215x over previous
"""Trainium2 Bass kernel for nn_Decoder_20486994002617.  v4.

8-core tensor-parallel 2-layer llama-style decoder with ragged token-merge
(handled on host), returning the masked-mean cross-entropy loss.

v2: fp8e4 DoubleRow for qkv / o / gate / up / down / lm_head, weights
pre-scaled (x64, up-proj x4) into e4m3 range, compensation folded into rope
constants and scaled PSUM->SBUF copies; host pre-chunks weights into SBUF
tile layout so streaming DMAs are contiguous.

v3 (latency restructure): split ARs, bridge xnt preps emitted inside the
previous phase's stream, fixed-max online softmax for the lm head.

v4 (ragged truncation + per-tile collectives):
  - Only the first valid_len tokens of the merged/compacted sequence can
    affect the loss (causal attention + masked CE), so the device kernel
    is compiled for T = ceil(valid_len/128) sequence tiles instead of the
    padded 8 (T=5 for the reference inputs).
  - AllReduces are per-tile (128 rows, ~1MB) and posted as soon as each
    tile's rows are ready, shrinking phase-boundary latency.
  - MLP runs in groups of <=4 tiles (PSUM accumulators); the last group's
    weight streaming hides the next phase's residual+norm+transpose preps.
"""
import numpy as np
import ml_dtypes

from contextlib import ExitStack

import concourse.bass as bass
import concourse.bacc as bacc
import concourse.mybir as mybir
import concourse.tile as tile
from concourse.bass_utils import run_bass_kernel_spmd

F32 = mybir.dt.float32
BF16 = mybir.dt.bfloat16
FP8 = mybir.dt.float8e4
AF = mybir.ActivationFunctionType
ALU = mybir.AluOpType
AX = mybir.AxisListType
DR = mybir.MatmulPerfMode.DoubleRow

H, HD, NH, NKV = 4096, 128, 32, 8
L, V, S, I = 2, 32000, 1024, 11008
EPS, THETA = 1e-6, 10000.0
NC_ = 8          # cores
IPC = I // NC_   # 1376
IP = 1408        # padded intermediate per core = 11 * 128
IP2 = 1536       # fp8-pair-padded contraction for down proj = 12 * 128
VS = V // NC_    # 4000 vocab per core
NEG = -1e9
WS = 64.0        # fp8 weight scale (qkv, o, gate, down, lm_head)
US = 4.0         # fp8 weight scale for y=silu(g)*u (y*US must stay under 240)
LM_MAX = 16.0    # fixed logsumexp shift (|logit| << 16)
SC_MAX = 12.0    # fixed softmax shift for attention scores (|score| << 12)
ALPHA = 16.0     # residual-stream prescale: raw h must stay inside fp8e4
                 # normals (max |h| ~ 11.9 over the run -> 190 < 240)

bf16 = ml_dtypes.bfloat16
f8 = ml_dtypes.float8_e4m3

last_run_info = {}
_cache = {}


# ----------------------------------------------------------------- device --

def _norm_transpose(nc, small, ntmp, psum, h_ap, dst, ident_sb, uid,
                    nt_tag="mix", nt_bufs=2):
    """dst[:, k, :] (32 chunks of [128,128]) = normalized transpose of
    h_ap ([128 seq rows, 4096]). dst free dims must be (32, 128)."""
    ssq = small.tile([128, 1], F32, tag="nt_ssq", bufs=2, name=f"ssq_{uid}")
    # Square scratch output goes into dst (overwritten by the transpose after)
    nc.scalar.activation(dst, h_ap.rearrange("p (k m) -> p k m", k=32),
                         AF.Square, accum_out=ssq[:])
    var = small.tile([128, 1], F32, tag="nt_var", bufs=2, name=f"var_{uid}")
    nc.vector.tensor_scalar(var[:], ssq[:], 1.0 / H, ALPHA * ALPHA * EPS,
                            op0=ALU.mult, op1=ALU.add)
    std = small.tile([128, 1], F32, tag="nt_std", bufs=2, name=f"std_{uid}")
    nc.scalar.sqrt(std[:], var[:])
    fac = small.tile([128, 1], F32, tag="nt_fac", bufs=2, name=f"fac_{uid}")
    nc.vector.reciprocal(fac[:], std[:])
    diag = ntmp.tile([128, 128], BF16, tag="nt_diag", bufs=2, name=f"diag_{uid}")
    nc.vector.tensor_scalar_mul(diag[:], ident_sb[:], fac[:])
    for kk in range(8):
        pnt = psum.tile([128, 512], F32, tag=nt_tag, bufs=nt_bufs,
                        name=f"pnt_{uid}_{kk}")
        for j in range(4):
            k = kk * 4 + j
            nc.tensor.matmul(pnt[:, j * 128:(j + 1) * 128],
                             h_ap[:, k * 128:(k + 1) * 128], diag[:],
                             start=True, stop=True)
        nc.any.tensor_copy(dst[:, kk * 4:(kk + 1) * 4, :],
                           pnt[:].rearrange("p (j m) -> p j m", j=4))


def _transpose_raw(nc, psum, h_ap, dst, ident_sb, uid, nt_tag="mix", nt_bufs=2):
    """dst[:, k, :] (32 chunks of [128,128]) = raw transpose of h_ap
    ([128 seq rows, 4096]); per-row norm factors are folded into the
    consumers' outputs instead (no vector dependency here)."""
    for kk in range(8):
        pnt = psum.tile([128, 512], F32, tag=nt_tag, bufs=nt_bufs,
                        name=f"pnt_{uid}_{kk}")
        for j in range(4):
            k = kk * 4 + j
            nc.tensor.matmul(pnt[:, j * 128:(j + 1) * 128],
                             h_ap[:, k * 128:(k + 1) * 128], ident_sb[:],
                             start=True, stop=True)
        nc.any.tensor_copy(dst[:, kk * 4:(kk + 1) * 4, :],
                           pnt[:].rearrange("p (j m) -> p j m", j=4))


def _norm_stats(nc, small, scrpool, h_ap, fac_ap, facws_ap, uid):
    """fac = 1/sqrt(mean(h^2)/1 + a^2 eps) per row; facws = fac/WS."""
    scr = scrpool.tile([128, 32, 128], BF16, tag="nt_scr", bufs=2,
                       name=f"scr_{uid}")
    ssq = small.tile([128, 1], F32, tag="nt_ssq", bufs=2, name=f"ssq_{uid}")
    nc.scalar.activation(scr[:], h_ap.rearrange("p (k m) -> p k m", k=32),
                         AF.Square, accum_out=ssq[:])
    var = small.tile([128, 1], F32, tag="nt_var", bufs=2, name=f"var_{uid}")
    nc.vector.tensor_scalar(var[:], ssq[:], 1.0 / H, ALPHA * ALPHA * EPS,
                            op0=ALU.mult, op1=ALU.add)
    std = small.tile([128, 1], F32, tag="nt_std", bufs=2, name=f"std_{uid}")
    nc.scalar.sqrt(std[:], var[:])
    nc.vector.reciprocal(fac_ap, std[:])
    nc.vector.tensor_scalar_mul(facws_ap, fac_ap, 1.0 / WS)


def _rope(nc, ntmp, ps, cos_ap, sf_ap, out, nheads, i):
    """out (bf16 [128, nheads*128]) = rope(ps); cos_ap/sf_ap are [128,128]."""
    n = nheads * 128
    t1 = ntmp.tile([128, 512], F32, tag="rope_t1", bufs=1, name=f"t1_{i}_{nheads}")
    t2 = ntmp.tile([128, 512], F32, tag="rope_t2", bufs=1, name=f"t2_{i}_{nheads}")
    for hh in range(nheads):
        b = hh * 128
        nc.vector.tensor_mul(t1[:, b:b + 128], ps[:, b:b + 128], cos_ap)
        nc.vector.tensor_mul(t2[:, b:b + 64], ps[:, b + 64:b + 128],
                             sf_ap[:, 0:64])
        nc.vector.tensor_mul(t2[:, b + 64:b + 128], ps[:, b:b + 64],
                             sf_ap[:, 64:128])
    nc.vector.tensor_add(out[:], t1[:, :n], t2[:, :n])


def build_nc(T):
    VLP = T * 128
    # MLP tile groups: keep the last group >=2 tiles so its re-streamed
    # weights amortize over enough GEMM work (compute-bound, not DMA-bound)
    split = min(4, max(1, T - 2))
    g0 = list(range(split))              # first MLP group (<=4 tiles)
    glast = list(range(split, T))        # last MLP group ([] if T <= 1)
    groups = [g0] + ([glast] if glast else [])

    nc = bacc.Bacc("TRN2", target_bir_lowering=False, debug=False,
                   num_devices=NC_)

    din = {}
    def dram_in(name, shape, dtype=BF16):
        din[name] = nc.dram_tensor(name, shape, dtype, kind="ExternalInput")
        return din[name]

    h0_d = dram_in("h0", [VLP, H])
    cos1_d = dram_in("cos1", [VLP, 128])
    sf1_d = dram_in("sf1", [VLP, 128])
    ident_d = dram_in("ident", [128, 128])
    cmask_d = dram_in("cmask", [128, 128])
    ones_d = dram_in("ones", [128, 1])
    for l in range(L):
        dram_in(f"qkvw{l}", [128, 32, 768], FP8)
        dram_in(f"ow{l}", [128, 4, H], FP8)
        dram_in(f"gw{l}", [3, 8, 128, 4, 512], FP8)   # [nb, kp, p, j, n]
        dram_in(f"uw{l}", [3, 8, 128, 4, 512], FP8)
        dram_in(f"dw{l}", [8, 3, 128, 4, 512], FP8)   # [n, tp, p, j, n]
    lmw_d = dram_in("lmw", [8, 8, 128, 4, 500], FP8)  # [vb, kp, p, j, n]
    wsel_d = dram_in("wsel", [H, VLP])

    gsum_o = nc.dram_tensor("gsum_o", [128, 2 * T], F32, kind="ExternalOutput")
    tlog_o = nc.dram_tensor("tlog_o", [1, VLP], F32, kind="ExternalOutput")

    rg = [list(range(NC_))]

    with tile.TileContext(nc) as tc:
        with (
            tc.tile_pool(name="pconst", bufs=1) as pconst,
            tc.tile_pool(name="psmall", bufs=1) as psmall,
            tc.tile_pool(name="pbridge", bufs=1) as pbridge,
            tc.tile_pool(name="pdram", bufs=1, space="DRAM") as pdram,
        ):
            ident_sb = pconst.tile([128, 128], BF16)
            cmask_sb = pconst.tile([128, 128], BF16)
            ones_sb = pconst.tile([128, 1], BF16)
            cos_sb = pconst.tile([128, T, 128], BF16)
            sf_sb = pconst.tile([128, T, 128], BF16)
            negSM = pconst.tile([128, 1], F32)
            nc.any.memset(negSM[:], -SC_MAX)
            nc.sync.dma_start(ident_sb[:], ident_d.ap())
            nc.sync.dma_start(cmask_sb[:], cmask_d.ap())
            nc.sync.dma_start(ones_sb[:], ones_d.ap())
            for i in range(T):
                nc.sync.dma_start(cos_sb[:, i, :], cos1_d.ap()[i * 128:(i + 1) * 128, :])
                nc.sync.dma_start(sf_sb[:, i, :], sf1_d.ap()[i * 128:(i + 1) * 128, :])

            xnt = [pbridge.tile([128, 32, 128], FP8, name=f"xnt_{j}")
                   for j in range(T)]
            fac_sbs = [pbridge.tile([128, 1], F32, name=f"fac_{j}")
                       for j in range(T)]
            facws_sbs = [pbridge.tile([128, 1], F32, name=f"facws_{j}")
                         for j in range(T)]

            hstack = ExitStack()
            phh = hstack.enter_context(tc.tile_pool(name="phh", bufs=1))
            h_sb = phh.tile([128, T, H], BF16)
            for i in range(T):
                nc.sync.dma_start(h_sb[:, i, :], h0_d.ap()[i * 128:(i + 1) * 128, :])

            # per-tile AR buffers: [T][128, H]
            ar_ins, ar_outss, ar2_ins, ar2_outss = [], [], [], []
            for l in range(L):
                ar_ins.append(pdram.tile([VLP, H], BF16, name=f"ar_in_{l}"))
                ar_outss.append([pdram.tile([128, H], BF16, addr_space="Shared",
                                            name=f"ar_out_{l}_{j}")
                                 for j in range(T)])
                ar2_ins.append(pdram.tile([VLP, H], BF16, name=f"ar2_in_{l}"))
                ar2_outss.append([pdram.tile([128, H], BF16, addr_space="Shared",
                                             name=f"ar2_out_{l}_{j}")
                                  for j in range(T)])

            def prep(pool, psum, j, res_t, dst, uid, nt_tag="mix", nt_bufs=2,
                     norm=False):
                """h_sb[:,j] += AR per-tile residual; dst = transpose.
                norm=True bakes the rms factor into dst (final xf path);
                otherwise dst is raw and fac/facws are computed on the side."""
                if res_t is not None:
                    rt = pool.tile([128, H], BF16, tag="prep_rt", bufs=2,
                                   name=f"rt_{uid}")
                    nc.sync.dma_start(rt[:], res_t[:])
                    nc.vector.tensor_add(h_sb[:, j, :], h_sb[:, j, :], rt[:])
                if norm:
                    _norm_transpose(nc, psmall, pool, psum, h_sb[:, j, :], dst,
                                    ident_sb, uid, nt_tag=nt_tag, nt_bufs=nt_bufs)
                else:
                    _transpose_raw(nc, psum, h_sb[:, j, :], dst, ident_sb,
                                   uid, nt_tag=nt_tag, nt_bufs=nt_bufs)
                    _norm_stats(nc, psmall, pool, h_sb[:, j, :],
                                fac_sbs[j][:], facws_sbs[j][:], uid)

            xfstack = ExitStack()
            xf_sb = None

            for l in range(L):
                # ======== attention: per-tile qkv -> heads -> o-proj ========
                with (
                    tc.tile_pool(name="pal", bufs=1) as pal,
                    tc.tile_pool(name="paps", bufs=1, space="PSUM") as paps,
                ):
                    kT_sb = pal.tile([128, VLP], BF16)
                    v_sb = pal.tile([128, T, 128], BF16)
                    ar_in = ar_ins[l]
                    ar_outs = ar_outss[l]
                    wqkv_sb = pal.tile([128, 32, 768], FP8)
                    ow_sb = pal.tile([128, 4, H], FP8)
                    nc.sync.dma_start(wqkv_sb[:], din[f"qkvw{l}"].ap())
                    nc.sync.dma_start(ow_sb[:], din[f"ow{l}"].ap())
                    if l == 0:
                        for j in range(T):
                            prep(pal, paps, j, None, xnt[j], f"i{j}")
                        late = []
                    else:
                        # tiles of the previous MLP's last group could not be
                        # prepped there (their ar2 posts at its very end)
                        late = list(glast)
                    qTs = {}
                    Ast = {}

                    def A_chunk(i, c):
                        """4 qkv pair-matmuls (k=4c..4c+3) for tile i; split
                        so chunks can fill the exp latency between heads."""
                        if c == 0:
                            Ast[i] = (
                                paps.tile([128, 512], F32, tag="psq", bufs=2,
                                          name=f"psq_{l}_{i}"),
                                paps.tile([128, 256], F32, tag="pskv", bufs=1,
                                          name=f"pskv_{l}_{i}"))
                        psq_t, pskv_t = Ast[i]
                        for k in range(4 * c, 4 * c + 4):
                            nc.tensor.matmul(psq_t[:], xnt[i][:, 2 * k:2 * k + 2, :],
                                             wqkv_sb[:, 2 * k:2 * k + 2, 0:512],
                                             start=(k == 0), stop=(k == 15),
                                             perf_mode=DR)
                            nc.tensor.matmul(pskv_t[:], xnt[i][:, 2 * k:2 * k + 2, :],
                                             wqkv_sb[:, 2 * k:2 * k + 2, 512:768],
                                             start=(k == 0), stop=(k == 15),
                                             perf_mode=DR)

                    def A_tail(i):
                        """rope + scales + transposes for tile i."""
                        psq_t, pskv_t = Ast.pop(i)
                        qT_sb = pal.tile([128, 4, 128], BF16, tag="qT",
                                         bufs=2, name=f"qT_{l}_{i}")
                        qTs[i] = qT_sb
                        q_rot = pal.tile([128, 512], BF16, tag="q_rot", bufs=2,
                                         name=f"qr_{l}_{i}")
                        k_rot = pal.tile([128, 128], BF16, tag="k_rot", bufs=2,
                                         name=f"kr_{l}_{i}")
                        _rope(nc, pal, psq_t[:], cos_sb[:, i, :], sf_sb[:, i, :],
                              q_rot, 4, f"{l}_{i}")
                        nc.vector.tensor_scalar_mul(q_rot[:], q_rot[:],
                                                    fac_sbs[i][:])
                        _rope(nc, pal, pskv_t[:, 0:128], cos_sb[:, i, :],
                              sf_sb[:, i, :], k_rot, 1, f"{l}_{i}")
                        nc.vector.tensor_scalar_mul(k_rot[:], k_rot[:],
                                                    fac_sbs[i][:])
                        nc.vector.tensor_scalar_mul(v_sb[:, i, :],
                                                    pskv_t[:, 128:256],
                                                    facws_sbs[i][:])
                        ptr = paps.tile([128, 512], F32, tag="mix", bufs=2,
                                        name=f"ptrq_{l}_{i}")
                        for hh in range(4):
                            nc.tensor.matmul(ptr[:, hh * 128:(hh + 1) * 128],
                                             q_rot[:, hh * 128:(hh + 1) * 128],
                                             ident_sb[:], start=True, stop=True)
                        nc.any.tensor_copy(qT_sb[:],
                                           ptr[:].rearrange("p (j m) -> p j m", j=4))
                        ptrk = paps.tile([128, 512], F32, tag="mix", bufs=2,
                                         name=f"ptrk_{l}_{i}")
                        nc.tensor.matmul(ptrk[:, :128], k_rot[:], ident_sb[:],
                                         start=True, stop=True)
                        nc.any.tensor_copy(kT_sb[:, i * 128:(i + 1) * 128], ptrk[:, :128])

                    for c in range(4):
                        A_chunk(0, c)
                    A_tail(0)
                    for i in range(T):
                        qT_sb = qTs.pop(i)
                        oT_sb = pal.tile([128, 4, 128], FP8, tag="oT",
                                         bufs=2, name=f"oT_{l}_{i}")
                        n2 = 128 * (i + 1)
                        for hh in range(4):
                            pss = paps.tile([128, VLP], F32, tag="pss", bufs=1,
                                            name=f"pss_{l}_{hh}_{i}")
                            lhs_q = qT_sb[:, hh, :]
                            c0 = 0
                            while c0 < n2 - 128:
                                N = min(512, n2 - 128 - c0)
                                nc.tensor.matmul(pss[:, c0:c0 + N], lhs_q,
                                                 kT_sb[:, c0:c0 + N],
                                                 start=True, stop=True)
                                c0 += N
                            nc.tensor.matmul(pss[:, n2 - 128:n2], lhs_q,
                                             kT_sb[:, n2 - 128:n2],
                                             start=True, stop=False)
                            nc.tensor.matmul(pss[:, n2 - 128:n2], ident_sb[:],
                                             cmask_sb[:], start=False, stop=True)
                            if i + 1 < T:
                                A_chunk(i + 1, hh)
                            sume = psmall.tile([128, 1], F32, tag="sume", bufs=2,
                                               name=f"sume_{l}_{hh}_{i}")
                            exp_sb = pal.tile([128, VLP], BF16, tag="exp", bufs=2,
                                              name=f"exp_{l}_{hh}_{i}")
                            nc.scalar.activation(exp_sb[:, :n2], pss[:, :n2], AF.Exp,
                                                 bias=negSM[:], accum_out=sume[:])
                            rec = psmall.tile([128, 1], F32, tag="rec", bufs=2,
                                              name=f"rec_{l}_{hh}_{i}")
                            nc.vector.reciprocal(rec[:], sume[:])
                            diag_r = pal.tile([128, 128], BF16, tag="diag_r", bufs=2,
                                              name=f"diagr_{l}_{hh}_{i}")
                            nc.vector.tensor_scalar_mul(diag_r[:], ident_sb[:], rec[:])
                            atcol = pal.tile([128, T, 128], BF16, tag="atcol", bufs=2,
                                             name=f"atcol_{l}_{hh}_{i}")
                            for jg in range(0, i + 1, 4):
                                ng = min(4, i + 1 - jg)
                                pat = paps.tile([128, 512], F32, tag="mix", bufs=2,
                                                name=f"pat_{l}_{hh}_{i}_{jg}")
                                for jj in range(ng):
                                    j = jg + jj
                                    nc.tensor.matmul(pat[:, jj * 128:(jj + 1) * 128],
                                                     exp_sb[:, j * 128:(j + 1) * 128],
                                                     diag_r[:], start=True, stop=True)
                                nc.any.tensor_copy(
                                    atcol[:, jg:jg + ng, :],
                                    pat[:, :ng * 128].rearrange("p (j m) -> p j m", j=ng))
                            pso = paps.tile([128, 128], F32, tag="pso", bufs=1,
                                            name=f"pso_{l}_{hh}_{i}")
                            for j in range(i + 1):
                                nc.tensor.matmul(pso[:], v_sb[:, j, :], atcol[:, j, :],
                                                 start=(j == 0), stop=(j == i))
                            nc.any.tensor_copy(oT_sb[:, hh, :], pso[:])
                        if i + 1 < T:
                            A_tail(i + 1)
                        ob = pal.tile([128, H], BF16, tag="ob", bufs=1,
                                      name=f"ob_{l}_{i}")
                        for n in range(8):
                            pps = paps.tile([128, 512], F32, tag="mix", bufs=2,
                                            name=f"pop_{l}_{i}_{n}")
                            for t in range(2):
                                nc.tensor.matmul(pps[:], oT_sb[:, 2 * t:2 * t + 2, :],
                                                 ow_sb[:, 2 * t:2 * t + 2,
                                                       n * 512:(n + 1) * 512],
                                                 start=(t == 0), stop=(t == 1),
                                                 perf_mode=DR)
                            nc.vector.tensor_scalar_mul(ob[:, n * 512:(n + 1) * 512],
                                                        pps[:], 1.0 / WS)
                        nc.sync.dma_start(ar_in[i * 128:(i + 1) * 128, :], ob[:])
                        nc.gpsimd.collective_compute(
                            "AllReduce", ALU.add, replica_groups=rg,
                            ins=[ar_in[i * 128:(i + 1) * 128, :].opt()],
                            outs=[ar_outs[i].opt()])
                        # prep this phase's late tiles (prev MLP tail ARs)
                        if l > 0 and 1 <= i <= len(late):
                            j = late[i - 1]
                            prep(pal, paps, j, ar2_outss[l - 1][j], xnt[j],
                                 f"a{l}_{j}")
                        # prep the first MLP tiles as their ARs land
                        if i == T - 2 and len(g0) > 0:
                            prep(pal, paps, 0, ar_outs[0], xnt[0], f"m{l}_0")
                        if i == T - 1 and len(g0) > 1:
                            prep(pal, paps, 1, ar_outs[1], xnt[1], f"m{l}_1")
                    for j in g0[2:]:
                        prep(pal, paps, j, ar_outs[j], xnt[j], f"m{l}_{j}")

                # ===== MLP: gate/up -> down in tile groups -> AR2 ===========
                if l == L - 1:
                    pxf = xfstack.enter_context(tc.tile_pool(name="pxf", bufs=1))
                    xf_sb = pxf.tile([128, 32, VLP], FP8)
                with (
                    tc.tile_pool(name="pml", bufs=1) as pml,
                    tc.tile_pool(name="pmps", bufs=1, space="PSUM") as pmps,
                ):
                    ar2_in = ar2_ins[l]
                    ar2_outs = ar2_outss[l]

                    def prep_next(j):
                        """prep for the next phase: attention l+1 (into xnt)
                        or the final norm (into xf)."""
                        if l < L - 1:
                            prep(pml, pmps, j, ar2_outs[j], xnt[j],
                                 f"a{l + 1}_{j}", nt_tag="mlpps", nt_bufs=4)
                        else:
                            prep(pml, pmps, j, ar2_outs[j],
                                 xf_sb[:, :, j * 128:(j + 1) * 128],
                                 f"f{j}", nt_tag="mlpps", nt_bufs=4, norm=True)

                    for gi, tiles_g in enumerate(groups):
                        glen = len(tiles_g)
                        # slots: work to interleave into this group's weight
                        # streaming (one item consumed per g/u nb-block)
                        if gi == 0 and glast:
                            # this phase's last-group tiles (attn ARs ready)
                            slots = [(j, ar_outss[l][j], xnt[j], f"m{l}_{j}")
                                     for j in glast]
                        elif gi == len(groups) - 1 and glast:
                            # next phase's first-group tiles (ar2 of group 0)
                            slots = [("next", j) for j in g0]
                        else:
                            slots = []
                        slots = list(slots)

                        def pop_slot():
                            if slots:
                                s = slots.pop(0)
                                if s[0] == "next":
                                    prep_next(s[1])
                                else:
                                    j, res_t, dst, uid = s
                                    prep(pml, pmps, j, res_t, dst, uid,
                                         nt_tag="mlpps", nt_bufs=4)

                        with tc.tile_pool(name=f"pgu{gi}", bufs=1) as pgu:
                            yt_sb = pml.tile([128, 12, glen * 128], FP8, tag="yt",
                                             bufs=2, name=f"yt_{l}_{gi}")
                            nc.vector.memset(yt_sb[:, 11, :], 0.0)
                            gu = {}
                            for wi, (wname, tag) in enumerate(
                                    ((f"gw{l}", "g"), (f"uw{l}", "u"))):
                                outs = [pgu.tile([128, IP], BF16, tag=tag, bufs=4,
                                                 name=f"{tag}_{l}_{gi}_{ii}")
                                        for ii in range(glen)]
                                gu[tag] = outs
                                for nb in range(3):
                                    NB = 512 if nb < 2 else IP - 1024
                                    pg = [pmps.tile([128, 512], F32, tag="mlpps", bufs=4,
                                                    name=f"pg_{l}_{gi}_{tag}_{nb}_{ii}")
                                          for ii in range(glen)]
                                    for kp in range(8):
                                        wt = pgu.tile([128, 4, 512], FP8, tag="wstream",
                                                      bufs=6,
                                                      name=f"wt_{l}_{gi}_{tag}_{nb}_{kp}")
                                        nc.sync.dma_start(wt[:], din[wname].ap()[nb, kp])
                                        for jp in range(2):
                                            k = kp * 4 + 2 * jp
                                            for ii in range(glen):
                                                nc.tensor.matmul(
                                                    pg[ii][:, :NB],
                                                    xnt[tiles_g[ii]][:, k:k + 2, :],
                                                    wt[:, 2 * jp:2 * jp + 2, :NB],
                                                    start=(k == 0), stop=(k == 30),
                                                    perf_mode=DR)
                                    for ii in range(glen):
                                        nc.vector.tensor_scalar_mul(
                                            outs[ii][:, nb * 512:nb * 512 + NB],
                                            pg[ii][:, :NB],
                                            fac_sbs[tiles_g[ii]][:])
                                    if not (wi == 0 and nb == 0):
                                        pop_slot()
                            pairs = [tiles_g[k:k + 2]
                                     for k in range(0, glen, 2)]
                            for iq, pair in enumerate(pairs):
                                for ii in pair:
                                    loc = tiles_g.index(ii)
                                    ysil = pgu.tile([128, IP], BF16, tag="ysil", bufs=2,
                                                    name=f"ysil_{l}_{ii}")
                                    nc.scalar.activation(ysil[:], gu["g"][loc][:], AF.Silu,
                                                         scale=1.0 / WS)
                                    y = gu["u"][loc]
                                    nc.vector.tensor_mul(y[:], ysil[:], y[:])
                                    for tq in range(3):
                                        ts = [tq * 4 + j for j in range(4) if tq * 4 + j < 11]
                                        ptr = pmps.tile([128, 512], F32, tag="mlpps", bufs=4,
                                                        name=f"ytr_{l}_{ii}_{tq}")
                                        for jj, t in enumerate(ts):
                                            nc.tensor.matmul(ptr[:, jj * 128:(jj + 1) * 128],
                                                             y[:, t * 128:(t + 1) * 128],
                                                             ident_sb[:], start=True, stop=True)
                                        nc.any.tensor_copy(
                                            yt_sb[:, ts[0]:ts[0] + len(ts),
                                                  loc * 128:(loc + 1) * 128],
                                            ptr[:, :len(ts) * 128].rearrange(
                                                "p (j m) -> p j m", j=len(ts)))
                                for n in range(8):
                                    pd = [pmps.tile([128, 512], F32, tag=f"pd{i2}", bufs=1,
                                                    name=f"pd_{l}_{gi}_{iq}_{n}_{i2}")
                                          for i2 in range(len(pair))]
                                    for tp in range(3):
                                        dwt = pgu.tile([128, 4, 512], FP8, tag="dwstream",
                                                       bufs=6,
                                                       name=f"dwt_{l}_{gi}_{iq}_{n}_{tp}")
                                        nc.sync.dma_start(dwt[:], din[f"dw{l}"].ap()[n, tp])
                                        for jp in range(2):
                                            c = tp * 4 + 2 * jp
                                            for i2, ii in enumerate(pair):
                                                loc = tiles_g.index(ii)
                                                nc.tensor.matmul(
                                                    pd[i2][:],
                                                    yt_sb[:, c:c + 2,
                                                          loc * 128:(loc + 1) * 128],
                                                    dwt[:, 2 * jp:2 * jp + 2, :],
                                                    start=(c == 0), stop=(c == 10),
                                                    perf_mode=DR)
                                    for i2, ii in enumerate(pair):
                                        db = pgu.tile([128, 512], BF16, tag="db", bufs=2,
                                                      name=f"db_{l}_{gi}_{iq}_{n}_{i2}")
                                        nc.vector.tensor_scalar_mul(db[:], pd[i2][:],
                                                                    1.0 / (WS * US))
                                        nc.sync.dma_start(
                                            ar2_in[ii * 128:(ii + 1) * 128,
                                                   n * 512:(n + 1) * 512], db[:])
                                for ii in pair:
                                    nc.gpsimd.collective_compute(
                                        "AllReduce", ALU.add, replica_groups=rg,
                                        ins=[ar2_in[ii * 128:(ii + 1) * 128, :].opt()],
                                        outs=[ar2_outs[ii].opt()])
                        if not glast and gi == len(groups) - 1:
                            # single-group fallback: emit all next-phase preps
                            for j in range(T):
                                prep_next(j)

                    if l == L - 1:
                        # final xf tiles of the last group (their ar2 lands
                        # late) with the tlog groups interleaved to hide the
                        # tail AR latency
                        with (
                            tc.tile_pool(name="ptl", bufs=1) as ptl,
                            tc.tile_pool(name="ptps", bufs=1, space="PSUM") as ptps,
                        ):
                            WA = len(g0) * 128
                            widths = [WA] + ([VLP - WA] if VLP > WA else [])
                            pts = [ptps.tile([1, w], F32, name=f"pt{h_}")
                                   for h_, w in enumerate(widths)]

                            def tlog_part(part, c0, W):
                                for kp in range(8):
                                    ws = ptl.tile([128, 4, W], BF16, tag="wsel",
                                                  bufs=2, name=f"ws_{part}_{kp}")
                                    nc.sync.dma_start(
                                        ws[:], wsel_d.ap()[kp * 512:(kp + 1) * 512,
                                                           c0:c0 + W]
                                        .rearrange("(j p) n -> p j n", p=128))
                                    for jk in range(4):
                                        k = kp * 4 + jk
                                        tm = ptl.tile([128, W], BF16, tag="tm",
                                                      bufs=2, name=f"tm_{part}_{k}")
                                        nc.vector.tensor_mul(
                                            tm[:], xf_sb[:, k, c0:c0 + W],
                                            ws[:, jk, :])
                                        nc.tensor.matmul(pts[part][:], ones_sb[:],
                                                         tm[:], start=(k == 0),
                                                         stop=(k == 31))

                            tlog_part(0, 0, WA)
                            for j in glast:
                                prep_next(j)
                            if len(widths) > 1:
                                tlog_part(1, WA, VLP - WA)
                            tl_sb = ptl.tile([1, VLP], F32)
                            nc.any.tensor_copy(tl_sb[:, :WA], pts[0][:])
                            if len(widths) > 1:
                                nc.any.tensor_copy(tl_sb[:, WA:], pts[1][:])
                            nc.sync.dma_start(tlog_o.ap(), tl_sb[:])

            # ==================== lm (online fixed-max softmax) =============

            with (
                tc.tile_pool(name="plm", bufs=1) as plm,
                tc.tile_pool(name="plps", bufs=1, space="PSUM") as plps,
                tc.tile_pool(name="pld", bufs=1, space="DRAM") as pld,
            ):
                # two partial-sum accumulators so the first AR (vocab blocks
                # 0..5) overlaps the last two blocks' compute
                s_sbs = [plm.tile([128, T], F32, name=f"s_sb{p}")
                         for p in range(2)]
                for p in range(2):
                    nc.any.memset(s_sbs[p][:], 0.0)
                negM = plm.tile([128, 1], F32)
                nc.any.memset(negM[:], -LM_MAX)
                gs_ins = [pld.tile([128, T], F32, name=f"gs_in{p}")
                          for p in range(2)]
                gs_outs = [pld.tile([128, T], F32, addr_space="Shared",
                                    name=f"gs_out{p}") for p in range(2)]
                for vb in range(8):
                    part = 0 if vb < 6 else 1
                    pl = [plps.tile([128, 500], F32, tag=f"pl{i}", bufs=1,
                                    name=f"pl_{vb}_{i}") for i in range(T)]
                    for kp in range(8):
                        lt = plm.tile([128, 4, 500], FP8, tag="lmw", bufs=6,
                                      name=f"lt_{vb}_{kp}")
                        nc.sync.dma_start(lt[:], lmw_d.ap()[vb, kp])
                        for jp in range(2):
                            k = kp * 4 + 2 * jp
                            for i in range(T):
                                nc.tensor.matmul(pl[i][:],
                                                 xf_sb[:, k:k + 2, i * 128:(i + 1) * 128],
                                                 lt[:, 2 * jp:2 * jp + 2, :],
                                                 start=(k == 0), stop=(k == 30),
                                                 perf_mode=DR)
                    for i in range(T):
                        se = psmall.tile([128, 1], F32, tag="se", bufs=2,
                                         name=f"se_{vb}_{i}")
                        scr = plm.tile([128, 500], BF16, tag="scr", bufs=2,
                                       name=f"scr_{vb}_{i}")
                        nc.scalar.activation(scr[:], pl[i][:], AF.Exp,
                                             bias=negM[:], scale=1.0 / WS,
                                             accum_out=se[:])
                        nc.vector.tensor_add(s_sbs[part][:, i:i + 1],
                                             s_sbs[part][:, i:i + 1], se[:])
                    if vb == 5 or vb == 7:
                        p = 0 if vb == 5 else 1
                        nc.sync.dma_start(gs_ins[p][:], s_sbs[p][:])
                        nc.gpsimd.collective_compute(
                            "AllReduce", ALU.add, replica_groups=rg,
                            ins=[gs_ins[p].opt()], outs=[gs_outs[p].opt()])
                gsf_sb = plm.tile([128, 2, T], F32)
                for p in range(2):
                    nc.sync.dma_start(gsf_sb[:, p, :], gs_outs[p][:])
                nc.sync.dma_start(gsum_o.ap(), gsf_sb[:])
            xfstack.close()
            hstack.close()

    nc.compile()
    return nc


# ------------------------------------------------------------------- host --

def _to_f8(x):
    return np.clip(x, -240.0, 240.0).astype(f8)


def host_prep(inputs):
    inp = {k: np.asarray(v) for k, v in inputs.items()}
    embed = inp["embed"].astype(np.float32)
    ids = inp["input_ids"].reshape(-1).astype(np.int64)
    labels = inp["labels"].reshape(-1).astype(np.int64)

    h = embed[ids]
    cw = inp["conv_w"].astype(np.float32)
    logit = h[:-1] @ cw[0, :H] + h[1:] @ cw[0, H:] + np.float32(inp["conv_b"][0])
    mask = logit > 0
    m = np.concatenate([mask, [False]])
    hn = np.where(m[:, None], 0.5 * (h + np.roll(h, -1, axis=0)), h)
    keep = np.concatenate([[True], ~mask])
    order = np.argsort(~keep, kind="stable")
    h0 = hn[order]
    lab = labels[order]
    valid_len = int(keep.sum())

    # ragged truncation: tokens >= valid_len cannot affect the masked loss
    T = (valid_len + 127) // 128
    VLP = T * 128
    h0 = h0[:VLP]

    inv = 1.0 / (THETA ** (np.arange(0, HD, 2, dtype=np.float32) / HD))
    t = np.arange(VLP, dtype=np.float32)
    freqs = np.outer(t, inv)
    emb = np.concatenate([freqs, freqs], -1)
    cos, sin = np.cos(emb), np.sin(emb)
    sinflip = np.concatenate([-sin[:, :HD // 2], sin[:, HD // 2:]], -1)
    # rope constants absorb the 1/WS compensation for the fp8 q/k weights
    cos1 = (cos / WS).astype(bf16)
    sf1 = (sinflip / WS).astype(bf16)

    ident = np.eye(128, dtype=bf16)
    cmask = np.where(np.arange(128)[None, :] > np.arange(128)[:, None],
                     np.float32(NEG), np.float32(0)).astype(bf16)
    ones = np.ones((128, 1), dtype=bf16)

    ln1 = inp["ln1_w"].astype(np.float32)
    ln2 = inp["ln2_w"].astype(np.float32)
    normw = inp["norm_w"].astype(np.float32)
    qsc = np.float32(1.0 / np.sqrt(HD))
    lm_folded = normw[:, None] * inp["lm_head_w"].astype(np.float32)
    lm_q = _to_f8(lm_folded * WS)          # quantized once, reused for wsel
    tgt = np.concatenate([lab[1:], [0]]).astype(np.int64)[:VLP]
    wsel = np.ascontiguousarray(lm_q.astype(np.float32)[:, tgt] / WS).astype(bf16)

    common = dict(h0=(h0 * ALPHA).astype(bf16), cos1=cos1, sf1=sf1,
                  ident=ident, cmask=cmask, ones=ones, wsel=wsel)
    in_maps = []
    for c in range(NC_):
        mcore = dict(common)
        for l in range(L):
            qw = ln1[l][:, None] * inp["q_w"][l].astype(np.float32) * qsc * WS
            kw = ln1[l][:, None] * inp["k_w"][l].astype(np.float32) * WS
            vw = ln1[l][:, None] * inp["v_w"][l].astype(np.float32) * WS
            gw = ln2[l][:, None] * inp["gate_w"][l].astype(np.float32) * WS
            uw = ln2[l][:, None] * inp["up_w"][l].astype(np.float32) * US
            dw = inp["down_w"][l].astype(np.float32) * (WS * ALPHA)
            qkv = np.concatenate(
                [qw[:, c * 512:(c + 1) * 512],
                 kw[:, c * 128:(c + 1) * 128],
                 vw[:, c * 128:(c + 1) * 128]], 1)          # [H, 768]
            mcore[f"qkvw{l}"] = np.ascontiguousarray(
                _to_f8(qkv).reshape(32, 128, 768).transpose(1, 0, 2))
            ow = inp["o_w"][l][c * 512:(c + 1) * 512].astype(np.float32) * (WS * ALPHA)
            mcore[f"ow{l}"] = np.ascontiguousarray(
                _to_f8(ow).reshape(4, 128, H).transpose(1, 0, 2))
            gws = np.zeros((H, IP), np.float32)
            uws = np.zeros((H, IP), np.float32)
            dws = np.zeros((IP2, H), np.float32)
            gws[:, :IPC] = gw[:, c * IPC:(c + 1) * IPC]
            uws[:, :IPC] = uw[:, c * IPC:(c + 1) * IPC]
            dws[:IPC] = dw[c * IPC:(c + 1) * IPC]
            for wname, warr in ((f"gw{l}", gws), (f"uw{l}", uws)):
                out = np.zeros((3, 8, 128, 4, 512), np.float32)
                for nb in range(3):
                    NBc = 512 if nb < 2 else IP - 1024
                    blk = warr[:, nb * 512:nb * 512 + NBc]       # [H, NBc]
                    out[nb, :, :, :, :NBc] = blk.reshape(
                        8, 4, 128, NBc).transpose(0, 2, 1, 3)
                mcore[wname] = _to_f8(out)
            dout = np.zeros((8, 3, 128, 4, 512), np.float32)
            for n in range(8):
                blk = dws[:, n * 512:(n + 1) * 512]              # [IP2, 512]
                dout[n] = blk.reshape(3, 4, 128, 512).transpose(0, 2, 1, 3)
            mcore[f"dw{l}"] = _to_f8(dout)
        lmc = lm_q[:, c * VS:(c + 1) * VS].astype(np.float32)    # [H, 4000]
        lout = np.zeros((8, 8, 128, 4, 500), np.float32)
        for vb in range(8):
            blk = lmc[:, vb * 500:(vb + 1) * 500]                # [H, 500]
            lout[vb] = blk.reshape(8, 4, 128, 500).transpose(0, 2, 1, 3)
        mcore["lmw"] = _to_f8(lout)
        in_maps.append(mcore)

    return in_maps, valid_len, T


def kernel(**inputs) -> np.ndarray:
    in_maps, valid_len, T = host_prep(inputs)
    VLP = T * 128
    if T not in _cache:
        _cache[T] = build_nc(T)
    nc = _cache[T]
    res = run_bass_kernel_spmd(nc, in_maps, list(range(NC_)),
                               **last_run_info.get("run_kwargs", {}))
    last_run_info["res"] = res
    out = res.results[0]
    gs = out["gsum_o"].reshape(128, 2, T).astype(np.float64)
    gsum = (gs[:, 0, :] + gs[:, 1, :]).transpose(1, 0).reshape(VLP)
    tlog = out["tlog_o"].reshape(VLP).astype(np.float64)
    ce = LM_MAX + np.log(gsum) - tlog
    loss = ce[:valid_len - 1].sum() / (valid_len - 1)
    return np.float32(loss)
